# revision 7
# baseline (speedup 1.0000x reference)
"""DRN layer kernel for 8 TRN2 NeuronCores.

Math (reference):
    T[j,k,l,m]   = exp(-w[j,k] * (s0[m]-s1[l])^2)
    Pw[i,j,k,l]  = sum_m T[j,k,l,m] * P[i,k,m]
    logsum[i,j,l]= sum_k log(Pw[i,j,k,l])
    out          = softmax_l(logsum + exponent_B[j,l])

Sharding: tensor-parallel over n_upper (j): 8 cores x 8 upper nodes each.
Every core sees the full batch. T is precomputed on the host (it only
depends on weight) and uploaded per-core in matmul-ready layout
[k, m, (j,l)]; exponent_B likewise. The device does, per k:
a [K=64, M=128, N=512] matmul per batch-half (Pw), a log (ScalarE),
and an accumulate (VectorE), then a softmax over l at the end.
"""

import numpy as np

B, NU, NL, QU, QL = 256, 64, 64, 64, 64
NCORES = 8
JLOC = NU // NCORES  # 8 upper nodes per core
JL = JLOC * QU       # 512 = packed (j, l) free dim


def _build_program():
    import concourse.bass as bass
    import concourse.bacc as bacc
    import concourse.mybir as mybir
    from concourse.tile import TileContext

    f32 = mybir.dt.float32
    AF = mybir.ActivationFunctionType

    nc = bacc.Bacc(None, target_bir_lowering=False)
    # PTT[k] = [P_k^T (256 cols) | T_k (512 cols)] packed so each k needs
    # one DMA (the fp32 self-loading matmul can only carry few sync waits).
    PTT = nc.declare_dram_parameter("PTT", [NL, QL, B + JL], f32, isOutput=False)
    EB = nc.declare_dram_parameter("EB", [128, JL], f32, isOutput=False)
    OUT = nc.declare_dram_parameter("out", [2, 128, JL], f32, isOutput=True)

    with TileContext(nc) as tc:
        with (
            # bufs=8 matches the 8-queue HWDGE round-robin: slot k is
            # rewritten by the DMA 8 iterations later, which lands on the
            # same HW queue, so WAW ordering is implicit (FIFO) and the
            # DMA stays within its 2-sync-wait ISA budget.
            tc.tile_pool(name="pt", bufs=8) as ppool,
            tc.tile_pool(name="ps", bufs=4, space="PSUM") as pspool,
            tc.tile_pool(name="lt", bufs=4) as lpool,
            tc.tile_pool(name="acc", bufs=1) as apool,
            tc.tile_pool(name="sm", bufs=4) as smpool,
            tc.tile_pool(name="ot", bufs=4) as opool,
        ):
            acc = []
            for ih in range(2):
                a = apool.tile([128, JL], f32, tag=f"acc{ih}")
                nc.sync.dma_start(out=a[:], in_=EB[:, :])
                acc.append(a)

            for k in range(NL):
                ptt = ppool.tile([QL, B + JL], f32, tag="ptt")
                nc.sync.dma_start(out=ptt[:], in_=PTT[k])
                for ih in range(2):
                    ps = pspool.tile([128, JL], f32, tag="ps")
                    nc.tensor.matmul(
                        ps[:],
                        lhsT=ptt[:, ih * 128:(ih + 1) * 128],
                        rhs=ptt[:, B:],
                        start=True,
                        stop=True,
                    )
                    lt = lpool.tile([128, JL], f32, tag="lt")
                    nc.scalar.activation(lt[:], ps[:], AF.Ln)
                    nc.vector.tensor_add(acc[ih][:], acc[ih][:], lt[:])

            for ih in range(2):
                for j in range(JLOC):
                    seg = acc[ih][:, j * QU:(j + 1) * QU]
                    negmx = smpool.tile([128, 1], f32, tag="negmx")
                    nc.vector.tensor_reduce(
                        negmx[:], seg, axis=mybir.AxisListType.X,
                        op=mybir.AluOpType.max, negate=True,
                    )
                    ex = opool.tile([128, QU], f32, tag="ex")
                    sm = smpool.tile([128, 1], f32, tag="sm")
                    nc.scalar.activation(
                        ex[:], seg, AF.Exp, bias=negmx[:], accum_out=sm[:],
                    )
                    rc = smpool.tile([128, 1], f32, tag="rc")
                    nc.vector.reciprocal(rc[:], sm[:])
                    ot = opool.tile([128, QU], f32, tag="ot")
                    nc.vector.tensor_scalar_mul(ot[:], ex[:], rc[:])
                    nc.sync.dma_start(
                        out=OUT[ih, :, j * QU:(j + 1) * QU], in_=ot[:],
                    )
    nc.compile()
    return nc


def _host_prep(P, weight, bias_abs, bias_q, lambda_abs, lambda_q):
    """Build per-core input maps. Host-side, cheap (T is 64MB total)."""
    s1 = (np.arange(QU, dtype=np.float32) / QU)          # upper bin centers
    s0 = (np.arange(QL, dtype=np.float32) / QL)          # lower bin centers
    diff2 = (s0[None, :] - s1[:, None]) ** 2             # [l, m]
    # T_full[j, k, l, m]
    T_full = np.exp(-weight[:, :, None, None].astype(np.float32)
                    * diff2[None, None, :, :].astype(np.float32))
    sq = s1
    expB = (-bias_q * (sq[None, :] - lambda_q) ** 2
            - bias_abs * np.abs(sq[None, :] - lambda_abs)).astype(np.float32)

    PT = P.transpose(1, 2, 0).astype(np.float32)         # [k, m, i]

    in_maps = []
    for c in range(NCORES):
        Tc = T_full[c * JLOC:(c + 1) * JLOC]             # [8, k, l, m]
        Tc = Tc.transpose(1, 3, 0, 2).reshape(NL, QL, JL)  # [k, m, (j,l)]
        PTTc = np.ascontiguousarray(
            np.concatenate([PT, Tc], axis=2))            # [k, m, 768]
        EBc = np.ascontiguousarray(np.broadcast_to(
            expB[c * JLOC:(c + 1) * JLOC].reshape(1, JL), (128, JL)))
        in_maps.append({"PTT": PTTc, "EB": EBc})
    return in_maps


_PROGRAM = None


def _get_program():
    global _PROGRAM
    if _PROGRAM is None:
        _PROGRAM = _build_program()
    return _PROGRAM


def run_on_device(in_maps, trace=False):
    from concourse.bass_utils import run_bass_kernel_spmd
    nc = _get_program()
    return run_bass_kernel_spmd(
        nc, in_maps, core_ids=list(range(NCORES)), trace=trace,
    )


def assemble(results):
    out = np.empty((B, NU, QU), dtype=np.float32)
    for c in range(NCORES):
        rc = results[c]["out"].reshape(B, JLOC, QU)
        out[:, c * JLOC:(c + 1) * JLOC, :] = rc
    return out


def kernel(P, weight, bias_abs, bias_q, lambda_abs, lambda_q):
    in_maps = _host_prep(P, weight, bias_abs, bias_q, lambda_abs, lambda_q)
    res = run_on_device(in_maps, trace=False)
    return assemble(res.results)


# revision 11
# speedup vs baseline: 2.3116x; 2.3116x over previous
"""DRN layer kernel for 8 TRN2 NeuronCores.

Math (reference):
    T[j,k,l,m]   = exp(-w[j,k] * (s0[m]-s1[l])^2)
    Pw[i,j,k,l]  = sum_m T[j,k,l,m] * P[i,k,m]
    logsum[i,j,l]= sum_k log(Pw[i,j,k,l])
    out          = softmax_l(logsum + exponent_B[j,l])

Sharding: tensor-parallel over n_upper (j): 8 cores x 8 upper nodes each.
Every core sees the full batch. T is precomputed on the host (it only
depends on weight) and uploaded per-core in matmul-ready layout
[k, m, (j,l)]; exponent_B likewise. The device does, per k:
a [K=64, M=128, N=512] matmul per batch-half (Pw), a log (ScalarE),
and an accumulate (VectorE), then a softmax over l at the end.
"""

import numpy as np

B, NU, NL, QU, QL = 256, 64, 64, 64, 64
NCORES = 8
JLOC = NU // NCORES  # 8 upper nodes per core
JL = JLOC * QU       # 512 = packed (j, l) free dim


def _build_program():
    import concourse.bass as bass
    import concourse.bacc as bacc
    import concourse.mybir as mybir
    from concourse.tile import TileContext

    f32 = mybir.dt.float32
    f32r = mybir.dt.float32r
    AF = mybir.ActivationFunctionType

    nc = bacc.Bacc(None, target_bir_lowering=False)
    # PTT[k] = [P_k^T (256 cols) | T_k (512 cols)] packed so each k needs
    # one DMA (the fp32 self-loading matmul can only carry few sync waits).
    PTT = nc.declare_dram_parameter("PTT", [NL, QL, B + JL], f32r, isOutput=False)
    EB = nc.declare_dram_parameter("EB", [128, JL], f32, isOutput=False)
    OUT = nc.declare_dram_parameter("out", [2, 128, JL], f32, isOutput=True)

    with TileContext(nc) as tc:
        with (
            # bufs=8 matches the 8-queue HWDGE round-robin: slot k is
            # rewritten by the DMA 8 iterations later, which lands on the
            # same HW queue, so WAW ordering is implicit (FIFO) and the
            # DMA stays within its 2-sync-wait ISA budget.
            tc.tile_pool(name="pt", bufs=8) as ppool,
            tc.tile_pool(name="ps", bufs=4, space="PSUM") as pspool,
            tc.tile_pool(name="lt", bufs=4) as lpool,
            tc.tile_pool(name="acc", bufs=1) as apool,
            tc.tile_pool(name="sm", bufs=4) as smpool,
            tc.tile_pool(name="ot", bufs=4) as opool,
        ):
            acc = []
            for ih in range(2):
                a = apool.tile([128, JL], f32, tag=f"acc{ih}")
                nc.sync.dma_start(out=a[:], in_=EB[:, :])
                acc.append(a)

            for k in range(NL):
                # float32r streams 1 row/cycle on the PE (plain fp32 is 4);
                # the whole PTT path is typed f32r so the verifier accepts it.
                ptt = ppool.tile([QL, B + JL], f32r, tag="ptt")
                nc.sync.dma_start(out=ptt[:], in_=PTT[k])
                for ih in range(2):
                    ps = pspool.tile([128, JL], f32, tag="ps")
                    nc.tensor.matmul(
                        ps[:],
                        lhsT=ptt[:, ih * 128:(ih + 1) * 128],
                        rhs=ptt[:, B:],
                        start=True,
                        stop=True,
                    )
                    lt = lpool.tile([128, JL], f32, tag="lt")
                    nc.scalar.activation(lt[:], ps[:], AF.Ln)
                    nc.vector.tensor_add(acc[ih][:], acc[ih][:], lt[:])

            for ih in range(2):
                for j in range(JLOC):
                    seg = acc[ih][:, j * QU:(j + 1) * QU]
                    negmx = smpool.tile([128, 1], f32, tag="negmx")
                    nc.vector.tensor_reduce(
                        negmx[:], seg, axis=mybir.AxisListType.X,
                        op=mybir.AluOpType.max, negate=True,
                    )
                    ex = opool.tile([128, QU], f32, tag="ex")
                    sm = smpool.tile([128, 1], f32, tag="sm")
                    nc.scalar.activation(
                        ex[:], seg, AF.Exp, bias=negmx[:], accum_out=sm[:],
                    )
                    rc = smpool.tile([128, 1], f32, tag="rc")
                    nc.vector.reciprocal(rc[:], sm[:])
                    ot = opool.tile([128, QU], f32, tag="ot")
                    nc.vector.tensor_scalar_mul(ot[:], ex[:], rc[:])
                    nc.sync.dma_start(
                        out=OUT[ih, :, j * QU:(j + 1) * QU], in_=ot[:],
                    )
    nc.compile()
    return nc


def _host_prep(P, weight, bias_abs, bias_q, lambda_abs, lambda_q):
    """Build per-core input maps. Host-side, cheap (T is 64MB total)."""
    s1 = (np.arange(QU, dtype=np.float32) / QU)          # upper bin centers
    s0 = (np.arange(QL, dtype=np.float32) / QL)          # lower bin centers
    diff2 = (s0[None, :] - s1[:, None]) ** 2             # [l, m]
    # T_full[j, k, l, m]
    T_full = np.exp(-weight[:, :, None, None].astype(np.float32)
                    * diff2[None, None, :, :].astype(np.float32))
    sq = s1
    expB = (-bias_q * (sq[None, :] - lambda_q) ** 2
            - bias_abs * np.abs(sq[None, :] - lambda_abs)).astype(np.float32)

    PT = P.transpose(1, 2, 0).astype(np.float32)         # [k, m, i]

    in_maps = []
    for c in range(NCORES):
        Tc = T_full[c * JLOC:(c + 1) * JLOC]             # [8, k, l, m]
        Tc = Tc.transpose(1, 3, 0, 2).reshape(NL, QL, JL)  # [k, m, (j,l)]
        PTTc = np.ascontiguousarray(
            np.concatenate([PT, Tc], axis=2))            # [k, m, 768]
        EBc = np.ascontiguousarray(np.broadcast_to(
            expB[c * JLOC:(c + 1) * JLOC].reshape(1, JL), (128, JL)))
        in_maps.append({"PTT": PTTc, "EB": EBc})
    return in_maps


_PROGRAM = None


def _get_program():
    global _PROGRAM
    if _PROGRAM is None:
        _PROGRAM = _build_program()
    return _PROGRAM


def run_on_device(in_maps, trace=False):
    from concourse.bass_utils import run_bass_kernel_spmd
    nc = _get_program()
    return run_bass_kernel_spmd(
        nc, in_maps, core_ids=list(range(NCORES)), trace=trace,
    )


def assemble(results):
    out = np.empty((B, NU, QU), dtype=np.float32)
    for c in range(NCORES):
        rc = results[c]["out"].reshape(B, JLOC, QU)
        out[:, c * JLOC:(c + 1) * JLOC, :] = rc
    return out


def kernel(P, weight, bias_abs, bias_q, lambda_abs, lambda_q):
    in_maps = _host_prep(P, weight, bias_abs, bias_q, lambda_abs, lambda_q)
    res = run_on_device(in_maps, trace=False)
    return assemble(res.results)
